# revision 11
# baseline (speedup 1.0000x reference)
"""Trainium2 Bass kernel for nn_AttEncode: 6-layer weight-shared encoder. V2.

Data-parallel over batch (B=32 -> 4 per core x 8 cores), zero collectives.
State kept transposed: hT0=[128(d),S], hT1=[73(d),S] (row 72 = ones, folds
every bias via K-augmented weights). Key structure vs V1:

- The layer stack is a hardware For_i loop (trip count is a build arg), so
  timing variants share an identical program and the wall-clock slope between
  trip counts measures pure device time.
- fuse fold: t = A@(qkv@F) with WF = [qkv_w; qkv_b]@fuse_sum precomputed
  host-side; the post-attention fuse matmul disappears.
- attention rowsum comes only from the ones-column of WF (row 72 of the
  second accumulator tile); no exp accum, no PE transposes of rowsums.
- residual adds ride scalar_tensor_tensor (u=(t+bias)+h) whose accum_out
  emits the LN row-sums for free; no PE identity matmuls.
- rstd = exp(-0.5*ln(var+eps)): every ACT func ({Exp,Ln,Square,Relu,Copy})
  lives in one activation table set -> zero LoadActFuncSet in steady state.
- scores/attention run in [*,512] PSUM pieces; PSUM = 4 banks generic pool
  + 2 banks attention accum + 2 banks smalls = exactly 8.
"""

import numpy as np
from contextlib import ExitStack

import concourse.bass as bass
import concourse.bass_isa as bass_isa
import concourse.tile as tile
from concourse import bacc, mybir
from concourse import bass_utils
from concourse.masks import make_identity
from concourse.bass import ts

# NOTE: every ACT function used here ({Exp, Ln, Square, Relu, Copy,
# Identity}) lives in the single table set natural_log_exp_and_others, so the
# loader hoists one LoadActFuncSet to the kernel entry. (Do NOT monkeypatch
# the table list down to one set: the runtime set-ID mapping breaks and Ln
# silently computes exp on hardware.)
B, S, D, H, STACK, V = 32, 1024, 200, 8, 6, 32000
N_CORES = 8
NB = B // N_CORES
NST = S // 128
D1 = D - 128               # 72
SCALE = 1.0 / float(np.sqrt(np.float32(D)))
INV_N = 1.0 / float(S * D)

F32 = mybir.dt.float32
F32R = mybir.dt.float32r
BF16 = mybir.dt.bfloat16
import os
BF16_ATT = os.environ.get('K_BF16','0') == '1'
F_STT = os.environ.get('K_STT','1') == '1'      # DVE scalar_tensor_tensor w/ accum
F_ACTN = os.environ.get('K_ACTN','1') == '1'    # ACT AP-scale normalize
F_PAR = os.environ.get('K_PAR','0') == '1'      # gpsimd partition all_reduce
F_PBC = os.environ.get('K_PBC', '1') == '1'  # gpsimd broadcast
F_DBG = os.environ.get('K_DBG', '0') == '1'  # debug dumps (batch 0, 1 layer)
F_DGA = os.environ.get('K_DGA', '0') == '1'  # dma_gather embed path (layout bug on HW; keep off)
I32 = mybir.dt.int32
AF = mybir.ActivationFunctionType
ALU = mybir.AluOpType
ENG = mybir.EngineType


def _r(ap):
    return ap.bitcast(F32R)


def _m(ap):
    # matmul operand view: f32r for f32 tiles, raw for bf16
    return ap if ap.dtype != F32 else ap.bitcast(F32R)


def build_nc(nb=NB, stack=STACK, io_reps=1, use_loop=True):
    nc = bacc.Bacc("TRN2", target_bir_lowering=False, debug=False,
                   enable_asserts=False)

    x_d = nc.dram_tensor("x", [nb, S], I32, kind="ExternalInput").ap()
    x16_d = nc.dram_tensor("x16", [nb, 16, S // 16], mybir.dt.int16,
                           kind="ExternalInput").ap()
    embed_d = nc.dram_tensor("embed", [V, 256], F32, kind="ExternalInput").ap()
    pos_d = nc.dram_tensor("pos", [S, D], F32, kind="ExternalInput").ap()
    wqa0_d = nc.dram_tensor("wqa0", [128, D], F32R, kind="ExternalInput").ap()
    wqa1_d = nc.dram_tensor("wqa1", [73, D], F32R, kind="ExternalInput").ap()
    wf0_d = nc.dram_tensor("wf0", [128, 256], F32R, kind="ExternalInput").ap()
    wf1_d = nc.dram_tensor("wf1", [73, 256], F32R, kind="ExternalInput").ap()
    w1a0_d = nc.dram_tensor("w1a0", [128, D + 1], F32R, kind="ExternalInput").ap()
    w1a1_d = nc.dram_tensor("w1a1", [73, D + 1], F32R, kind="ExternalInput").ap()
    w2a0_d = nc.dram_tensor("w2a0", [128, D], F32R, kind="ExternalInput").ap()
    w2a1_d = nc.dram_tensor("w2a1", [73, D], F32R, kind="ExternalInput").ap()
    bf_d = nc.dram_tensor("bf", [D, 1], F32, kind="ExternalInput").ap()
    cones_d = nc.dram_tensor("cones", [1, S], F32R, kind="ExternalInput").ap()
    y_d = nc.dram_tensor("y", [nb, S, D], F32, kind="ExternalOutput").ap()
    if F_DBG:
        dbg_u = nc.dram_tensor("dbg_u", [128, S], F32, kind="ExternalOutput").ap()
        dbg_h = nc.dram_tensor("dbg_h", [128, S], F32, kind="ExternalOutput").ap()
        dbg_f = nc.dram_tensor("dbg_f", [128, S], F32, kind="ExternalOutput").ap()
        dbg_u2 = nc.dram_tensor("dbg_u2", [128, S], F32, kind="ExternalOutput").ap()
        dbg_qT = nc.dram_tensor("dbg_qT", [128, S], F32, kind="ExternalOutput").ap()
        dbg_c = nc.dram_tensor("dbg_c", [128, S], F32, kind="ExternalOutput").ap()

    with tile.TileContext(nc) as tc, ExitStack() as ctx:
        const = ctx.enter_context(tc.tile_pool(name="const", bufs=1))
        state = ctx.enter_context(tc.tile_pool(name="state", bufs=1))
        p_qkvT = ctx.enter_context(tc.tile_pool(name="p_qkvT", bufs=2))
        p_qf = ctx.enter_context(tc.tile_pool(name="p_qf", bufs=4))
        p_E = ctx.enter_context(tc.tile_pool(name="p_E", bufs=16 if BF16_ATT else 8))
        p_t1 = ctx.enter_context(tc.tile_pool(name="p_t1", bufs=4))
        p_B = ctx.enter_context(tc.tile_pool(name="p_B", bufs=4))
        p_row = ctx.enter_context(tc.tile_pool(name="p_row", bufs=4))
        p_u = ctx.enter_context(tc.tile_pool(name="p_u", bufs=2))
        p_f1 = ctx.enter_context(tc.tile_pool(name="p_f1", bufs=2))
        p_scr = ctx.enter_context(tc.tile_pool(name="p_scr", bufs=2))
        p_sm = ctx.enter_context(tc.tile_pool(name="p_sm", bufs=4))
        p_emb = ctx.enter_context(tc.tile_pool(name="p_emb", bufs=2))
        p_gall = ctx.enter_context(tc.tile_pool(name="p_gall", bufs=1))
        psM = ctx.enter_context(tc.tile_pool(name="psM", bufs=2, space="PSUM"))
        psSm = ctx.enter_context(tc.tile_pool(name="psSm", bufs=2, space="PSUM"))
        psC = ctx.enter_context(tc.tile_pool(name="psC", bufs=2, space="PSUM"))

        # ---- constants & weights ----
        id128 = const.tile([128, 128], F32, tag="id128")
        make_identity(nc, id128[:])
        ones_col = const.tile([128, 1], F32R, tag="ones_col")
        nc.sync.dma_start(ones_col[:], cones_d[0, 0:128].rearrange("(p one) -> p one", one=1))
        ones_row = const.tile([1, 128], F32R, tag="ones_row")
        nc.sync.dma_start(ones_row[:], cones_d[0:1, 0:128])
        eps_ap = const.tile([1, 1], F32, tag="eps")
        nc.vector.memset(eps_ap[:], 1e-5)
        magic = const.tile([1, 1], I32, tag="magic")
        nc.vector.memset(magic[:], 0x5f3759df)

        wqa0 = const.tile([128, D], F32R, tag="wqa0")
        nc.sync.dma_start(wqa0[:], wqa0_d[:])
        wqa1 = const.tile([73, D], F32R, tag="wqa1")
        nc.sync.dma_start(wqa1[:], wqa1_d[:])
        wf0 = const.tile([128, 256], F32R, tag="wf0")
        nc.sync.dma_start(wf0[:], wf0_d[:])
        wf1 = const.tile([73, 256], F32R, tag="wf1")
        nc.sync.dma_start(wf1[:], wf1_d[:])
        w1a0 = const.tile([128, D + 1], F32R, tag="w1a0")
        nc.sync.dma_start(w1a0[:], w1a0_d[:])
        w1a1 = const.tile([73, D + 1], F32R, tag="w1a1")
        nc.sync.dma_start(w1a1[:], w1a1_d[:])
        w2a0 = const.tile([128, D], F32R, tag="w2a0")
        nc.sync.dma_start(w2a0[:], w2a0_d[:])
        w2a1 = const.tile([73, D], F32R, tag="w2a1")
        nc.sync.dma_start(w2a1[:], w2a1_d[:])
        bf0 = const.tile([128, 1], F32, tag="bf0")
        nc.sync.dma_start(bf0[:], bf_d[0:128, :])
        bf1 = const.tile([D1, 1], F32, tag="bf1")
        nc.sync.dma_start(bf1[:], bf_d[128:D, :])

        pos_t = []
        for st in range(NST):
            pt = const.tile([128, D], F32, tag=f"pos{st}")
            nc.sync.dma_start(pt[:], pos_d[ts(st, 128), :])
            pos_t.append(pt)

        hT0, hT1 = [], []
        for b in range(nb):
            t0 = state.tile([128, S], F32, tag=f"hT0_{b}")
            t1 = state.tile([73, S], F32, tag=f"hT1_{b}")
            nc.sync.dma_start(_r(t1[72:73, :]), cones_d[0:1, :])
            hT0.append(t0)
            hT1.append(t1)

        # ---- embedding gather + pos + transpose into hT ----
        def embed_body(_i=None):
            for b in range(nb):
                if F_DGA:
                    idx16 = p_emb.tile([16, S // 16], mybir.dt.int16, tag="idx16")
                    nc.sync.dma_start(idx16[:], x16_d[b])
                    gall = p_gall.tile([128, NST, 256], F32, tag="gall")
                    nc.gpsimd.dma_gather(
                        out_ap=gall[:], in_ap=embed_d[:], idxs_ap=idx16[:],
                        num_idxs=S, num_idxs_reg=S, elem_size=256)
                else:
                    idx = p_emb.tile([128, NST], I32, tag="idx")
                    nc.sync.dma_start(idx[:], x_d[b].rearrange("(t p) -> p t", p=128))
                for st in range(NST):
                    if F_DGA:
                        g = gall[:, st, 0:D]
                    else:
                        gt = p_emb.tile([128, 256], F32, tag="g")
                        nc.gpsimd.indirect_dma_start(
                            out=gt[:], out_offset=None, in_=embed_d[:],
                            in_offset=bass.IndirectOffsetOnAxis(ap=idx[:, st:st + 1], axis=0),
                        )
                        g = gt[:, 0:D]
                    h0 = p_emb.tile([128, D], F32, tag="h0")
                    nc.vector.tensor_add(h0[:], g, pos_t[st][:])
                    tr0 = psSm.tile([128, 128], F32, tag="sm")
                    nc.tensor.transpose(tr0[:], h0[:, 0:128], id128[:])
                    nc.scalar.copy(_r(hT0[b][:, ts(st, 128)]), tr0[:])
                    tr1 = psSm.tile([D1, 128], F32, tag="sm")
                    nc.tensor.transpose(tr1[:], h0[:, 128:D], id128[:])
                    nc.scalar.copy(_r(hT1[b][0:D1, ts(st, 128)]), tr1[:])

        # ---- one full layer sweep over all batches (phase-major) ----
        def layer_body(_i=None):
            # P3: software pipeline: stage A computes qkvT/qkvF/scores for
            # batch b while stage B consumes batch b-1 (attention + LN1), so
            # the PE always has stage-A matmuls during B's normalize chain.
            stash = {}

            def stage_a(b):
                h0t, h1t = hT0[b], hT1[b]
                qkvT0 = p_qkvT.tile([128, S], F32, tag="qkvT0")
                qkvT1 = p_qkvT.tile([D1, S], F32, tag="qkvT1")
                for (mt, msz, wsl) in ((qkvT0, 128, slice(0, 128)),
                                       (qkvT1, D1, slice(128, D))):
                    for n2 in range(2):
                        nsl = ts(n2, 512)
                        ps = psM.tile([msz, 512], F32, tag="mm")
                        nc.tensor.matmul(ps[:], _r(wqa0[:, wsl]), _r(h0t[:, nsl]),
                                         start=True, stop=False)
                        nc.tensor.matmul(ps[:], _r(wqa1[:, wsl]), _r(h1t[:, nsl]),
                                         start=False, stop=True)
                        nc.vector.tensor_copy(_r(mt[:, nsl]), ps[:])
                qf = []
                for g2 in range(2):
                    sb = p_qf.tile([128, S], BF16 if BF16_ATT else F32, tag="qf")
                    for q2 in range(2):
                        ps = psM.tile([128, 512], F32, tag="mm")
                        for k2 in range(2):
                            st = g2 * 4 + q2 * 2 + k2
                            reg = slice(k2 * 256, (k2 + 1) * 256)
                            nc.tensor.matmul(ps[:, reg], _r(h0t[:, ts(st, 128)]),
                                             _r(wf0[:]), start=True, stop=False)
                            nc.tensor.matmul(ps[:, reg], _r(h1t[:, ts(st, 128)]),
                                             _r(wf1[:]), start=False, stop=True)
                        nc.vector.tensor_copy(sb[:, ts(q2, 512)] if sb.dtype != F32 else _r(sb[:, ts(q2, 512)]), ps[:])
                    qf.append(sb)
                E = []
                for qt in range(NST):
                    e = p_E.tile([128, S], BF16 if BF16_ATT else F32, tag="E")
                    for h2 in range(2):
                        nsl = ts(h2, 512)
                        ps = psM.tile([128, 512], F32, tag="mm")
                        nc.tensor.matmul(ps[:], _r(qkvT0[:, ts(qt, 128)]),
                                         _r(qkvT0[:, nsl]), start=True, stop=False)
                        nc.tensor.matmul(ps[:], _r(qkvT1[:, ts(qt, 128)]),
                                         _r(qkvT1[:, nsl]), start=False, stop=True)
                        nc.scalar.activation(e[:, nsl] if e.dtype != F32 else _r(e[:, nsl]), ps[:], AF.Exp,
                                             bias=0.0, scale=SCALE)
                    E.append(e)
                if F_DBG and b == 0:
                    nc.sync.dma_start(dbg_qT[:], qkvT0[:])
                stash[b] = (qf, E)

            def stage_b(b):
                h0t, h1t = hT0[b], hT1[b]
                qf, E = stash.pop(b)
                u0 = p_u.tile([128, S], F32, tag="u0")
                u1 = p_u.tile([D1, S], F32, tag="u1")
                sacc = []
                for h2 in range(2):
                    nsl = ts(h2, 512)
                    pc0 = psC.tile([128, 512], F32, tag="c0")
                    pc1 = psC.tile([97, 512], F32, tag="c1")
                    for kt in range(NST):
                        g2, j = kt // 4, kt % 4
                        base = j * 256
                        nc.tensor.matmul(pc0[:], _m(qf[g2][:, base:base + 128]),
                                         _m(E[kt][:, nsl]),
                                         start=(kt == 0), stop=(kt == NST - 1))
                        nc.tensor.matmul(pc1[:], _m(qf[g2][:, base + 128:base + 225]),
                                         _m(E[kt][:, nsl]),
                                         start=(kt == 0), stop=(kt == NST - 1))
                    rr = p_row.tile([1, 512], F32, tag="rr")
                    with nc.allow_low_precision(reason="fp32r rounding of 1/rowsum"):
                        nc.vector.reciprocal(_r(rr[:]), pc1[96:97, :])
                    Bs = p_B.tile([128, 512], F32, tag="Bsb")
                    if F_PBC:
                        nc.gpsimd.partition_broadcast(Bs[:], rr[:])
                    else:
                        psB = psM.tile([128, 512], F32, tag="mm")
                        nc.tensor.matmul(psB[:], _r(ones_row[:]), _r(rr[:]),
                                         start=True, stop=True)
                        nc.scalar.copy(Bs[:], psB[:])
                    t10 = p_t1.tile([128, 512], F32, tag="t10")
                    t11 = p_t1.tile([D1, 512], F32, tag="t11")
                    nc.vector.tensor_mul(t10[:], pc0[:], Bs[:])
                    if F_DBG and b == 0:
                        nc.sync.dma_start(dbg_c[:, nsl], t10[:])
                    nc.vector.tensor_mul(t11[:], pc1[0:D1, :], Bs[0:D1, :])
                    s0 = p_sm.tile([128, 1], F32, tag="sacc")
                    s1 = p_sm.tile([D1, 1], F32, tag="sacc1")
                    if F_STT:
                        nc.vector.scalar_tensor_tensor(
                            out=u0[:, nsl], in0=t10[:], scalar=bf0[:],
                            in1=h0t[:, nsl], op0=ALU.add, op1=ALU.add, accum_out=s0[:])
                        nc.vector.scalar_tensor_tensor(
                            out=u1[:, nsl], in0=t11[:], scalar=bf1[:],
                            in1=h1t[0:D1, nsl], op0=ALU.add, op1=ALU.add, accum_out=s1[:])
                    else:
                        nc.vector.tensor_scalar(out=t10[:], in0=t10[:], scalar1=bf0[:],
                                                scalar2=None, op0=ALU.add)
                        nc.vector.tensor_add(u0[:, nsl], t10[:], h0t[:, nsl])
                        nc.vector.tensor_reduce(out=s0[:], in_=u0[:, nsl],
                                                axis=mybir.AxisListType.X, op=ALU.add)
                        nc.vector.tensor_scalar(out=t11[:], in0=t11[:], scalar1=bf1[:],
                                                scalar2=None, op0=ALU.add)
                        nc.vector.tensor_add(u1[:, nsl], t11[:], h1t[0:D1, nsl])
                        nc.vector.tensor_reduce(out=s1[:], in_=u1[:, nsl],
                                                axis=mybir.AxisListType.X, op=ALU.add)
                    sacc.append((s0, s1))
                if F_DBG and b == 0:
                    nc.sync.dma_start(dbg_u[:], u0[:])
                pq0 = p_sm.tile([128, 2], F32, tag="pq0")
                pq1 = p_sm.tile([D1, 2], F32, tag="pq1")
                with nc.allow_low_precision(reason="fp32r rounding of LN sums"):
                    nc.vector.tensor_add(_r(pq0[:, 0:1]), sacc[0][0][:], sacc[1][0][:])
                    nc.vector.tensor_add(_r(pq1[:, 0:1]), sacc[0][1][:], sacc[1][1][:])
                self_ln(nc, p_scr, p_sm, psSm, ones_col, ones_row, eps_ap, magic,
                        u0, u1, pq0, pq1, h0t, h1t)
                if F_DBG and b == 0:
                    nc.sync.dma_start(dbg_h[:], h0t[:])

            for b in range(nb):
                stage_a(b)
                if b > 0:
                    stage_b(b - 1)
            stage_b(nb - 1)

            # P4: per batch: FFN + LN2
            for b in range(nb):
                h0t, h1t = hT0[b], hT1[b]
                f10 = p_f1.tile([128, S], F32, tag="f10")
                f11 = p_f1.tile([73, S], F32, tag="f11")
                for (ft, msl) in ((f10, slice(0, 128)), (f11, slice(128, D + 1))):
                    msz = msl.stop - msl.start
                    for n2 in range(2):
                        nsl = ts(n2, 512)
                        ps = psM.tile([msz, 512], F32, tag="mm")
                        nc.tensor.matmul(ps[:], _r(w1a0[:, msl]), _r(h0t[:, nsl]),
                                         start=True, stop=False)
                        nc.tensor.matmul(ps[:], _r(w1a1[:, msl]), _r(h1t[:, nsl]),
                                         start=False, stop=True)
                        nc.scalar.activation(_r(ft[:, nsl]), ps[:], AF.Relu,
                                             bias=0.0, scale=1.0)

                u20 = p_u.tile([128, S], F32, tag="u0")
                u21 = p_u.tile([D1, S], F32, tag="u1")
                pq0f = p_sm.tile([128, 2], F32, tag="pq0")
                pq1f = p_sm.tile([D1, 2], F32, tag="pq1")
                sacc2 = []
                for n2 in range(2):
                    nsl = ts(n2, 512)
                    pg0 = psM.tile([128, 512], F32, tag="mm")
                    pg1 = psM.tile([D1, 512], F32, tag="mm")
                    for (pg, wsl) in ((pg0, slice(0, 128)), (pg1, slice(128, D))):
                        nc.tensor.matmul(pg[:], _r(w2a0[:, wsl]), _r(f10[:, nsl]),
                                         start=True, stop=False)
                        nc.tensor.matmul(pg[:], _r(w2a1[:, wsl]), _r(f11[:, nsl]),
                                         start=False, stop=True)
                    s0 = p_sm.tile([128, 1], F32, tag="sacc")
                    s1 = p_sm.tile([D1, 1], F32, tag="sacc1")
                    with nc.allow_low_precision(reason="fp32r rounding of LN sums"):
                    if F_STT:
                        nc.vector.scalar_tensor_tensor(
                            out=u20[:, nsl], in0=pg0[:], scalar=0.0,
                            in1=h0t[:, nsl], op0=ALU.add, op1=ALU.add,
                            accum_out=s0[:])
                        nc.vector.scalar_tensor_tensor(
                            out=u21[:, nsl], in0=pg1[:], scalar=0.0,
                            in1=h1t[0:D1, nsl], op0=ALU.add, op1=ALU.add,
                            accum_out=s1[:])
                    else:
                        nc.vector.tensor_add(u20[:, nsl], pg0[:], h0t[:, nsl])
                        nc.vector.tensor_reduce(out=s0[:], in_=u20[:, nsl],
                                                axis=mybir.AxisListType.X, op=ALU.add)
                        nc.vector.tensor_add(u21[:, nsl], pg1[:], h1t[0:D1, nsl])
                        nc.vector.tensor_reduce(out=s1[:], in_=u21[:, nsl],
                                                axis=mybir.AxisListType.X, op=ALU.add)
                    sacc2.append((s0, s1))
                if F_DBG and b == 0:
                    nc.sync.dma_start(dbg_f[:], f10[:])
                    nc.sync.dma_start(dbg_u2[:], u20[:])
                with nc.allow_low_precision(reason="fp32r rounding of LN sums"):
                    nc.vector.tensor_add(_r(pq0f[:, 0:1]), sacc2[0][0][:], sacc2[1][0][:])
                    nc.vector.tensor_add(_r(pq1f[:, 0:1]), sacc2[0][1][:], sacc2[1][1][:])
                self_ln(nc, p_scr, p_sm, psSm, ones_col, ones_row, eps_ap, magic,
                        u20, u21, pq0f, pq1f, h0t, h1t)

        # ---- output: transpose hT back to [s, d] and DMA out ----
        def out_body(_i=None):
            for b in range(nb):
                for st in range(NST):
                    o = p_emb.tile([128, D], F32, tag="o")
                    tr0 = psSm.tile([128, 128], F32, tag="sm")
                    nc.tensor.transpose(tr0[:], hT0[b][:, ts(st, 128)], id128[:])
                    nc.scalar.copy(o[:, 0:128], tr0[:])
                    tr1 = psSm.tile([128, D1], F32, tag="sm")
                    nc.tensor.transpose(tr1[:], hT1[b][0:D1, ts(st, 128)],
                                        id128[0:D1, 0:D1])
                    nc.scalar.copy(o[:, 128:D], tr1[:])
                    nc.sync.dma_start(y_d[b, ts(st, 128), :], o[:])

        if use_loop:
            with tc.For_i(0, io_reps, 1, hint_engines=(ENG.PE,)):
                embed_body()
            with tc.For_i(0, stack, 1, hint_engines=(ENG.PE,)):
                layer_body()
            with tc.For_i(0, io_reps, 1, hint_engines=(ENG.PE,)):
                out_body()
        else:
            for _ in range(io_reps):
                embed_body()
            for _ in range(stack):
                layer_body()
            for _ in range(io_reps):
                out_body()

    nc.compile()
    return nc


def self_ln(nc, p_scr, p_sm, psSm, ones_col, ones_row, eps_ap, magic,
            u0, u1, pq0, pq1, h0t, h1t):
    """Joint layernorm over (S, D) of u (SBUF); writes back into hT.

    pq0/pq1 arrive with col 0 = per-partition row sums (from STT accum).
    Entirely PE-free: cross-partition reduction and broadcast run on Pool.
    """
    with nc.allow_low_precision(reason="fp32r rounding of LN partial sums"):
        scr0 = p_scr.tile([128, S], F32, tag="scr0")
        nc.scalar.activation(scr0[:], u0[:], AF.Square, accum_out=_r(pq0[:, 1:2]))
        scr1 = p_scr.tile([D1, S], F32, tag="scr1")
        nc.scalar.activation(scr1[:], u1[:], AF.Square, accum_out=_r(pq1[:, 1:2]))

    if F_PAR:
        ar0 = p_sm.tile([128, 2], F32, tag="ar0")
        nc.gpsimd.partition_all_reduce(ar0[:], pq0[:], 128, bass_isa.ReduceOp.add)
        ar1 = p_sm.tile([D1, 2], F32, tag="ar1")
        nc.gpsimd.partition_all_reduce(ar1[:], pq1[:], D1, bass_isa.ReduceOp.add)
        stp = p_sm.tile([1, 2], F32, tag="stp")
        nc.vector.tensor_add(stp[:], ar0[0:1, :], ar1[0:1, :])
    else:
        stpp = psSm.tile([1, 2], F32, tag="sm")
        nc.tensor.matmul(stpp[:], _r(ones_col[:]), _r(pq0[:]), start=True, stop=False)
        nc.tensor.matmul(stpp[:], _r(ones_col[0:D1, :]), _r(pq1[:]), start=False, stop=True)
        stp = p_sm.tile([1, 2], F32, tag="stp")
        nc.vector.tensor_copy(stp[:], stpp[:])

    stat = p_sm.tile([1, 2], F32, tag="stat")     # [mean, meansq]
    nc.scalar.mul(stat[:], stp[:], INV_N)
    msq = p_sm.tile([1, 1], F32, tag="msq")
    nc.scalar.activation(msq[:], stat[:, 0:1], AF.Square)
    var = p_sm.tile([1, 1], F32, tag="var")
    nc.vector.tensor_sub(var[:], stat[:, 1:2], msq[:])
    # rstd = rsqrt(var + eps) via bit-trick + 2 Newton steps, all on DVE
    # (table-free: keeps ACT pinned to the exp_and_others function set)
    pk = p_sm.tile([1, 2], F32, tag="pk")         # [rstd, -mean*rstd]
    ve = p_sm.tile([1, 1], F32, tag="ve")
    nc.vector.tensor_scalar(out=ve[:], in0=var[:], scalar1=eps_ap[:],
                            scalar2=None, op0=ALU.add)
    vh = p_sm.tile([1, 1], F32, tag="vh")
    nc.vector.tensor_scalar(out=vh[:], in0=ve[:], scalar1=0.5,
                            scalar2=None, op0=ALU.mult)
    yg = p_sm.tile([1, 1], I32, tag="yg")
    nc.vector.tensor_scalar(out=yg[:], in0=ve[:].bitcast(I32), scalar1=1,
                            scalar2=None, op0=ALU.logical_shift_right)
    yi = p_sm.tile([1, 1], I32, tag="yi")
    nc.vector.tensor_sub(yi[:], magic[:], yg[:])
    cur = yi[:].bitcast(F32)
    for it in range(2):
        aa = p_sm.tile([1, 1], F32, tag=f"nra{it}")
        nc.vector.tensor_mul(aa[:], cur, cur)
        bb = p_sm.tile([1, 1], F32, tag=f"nrb{it}")
        nc.vector.tensor_mul(bb[:], aa[:], vh[:])
        cc = p_sm.tile([1, 1], F32, tag=f"nrc{it}")
        nc.vector.tensor_scalar(out=cc[:], in0=bb[:], scalar1=-1.0, scalar2=1.5,
                                op0=ALU.mult, op1=ALU.add)
        dd = p_sm.tile([1, 1], F32, tag=f"nrd{it}")
        out_ap = _r(pk[:, 0:1]) if it == 1 else dd[:]
        with nc.allow_low_precision(reason="fp32r rounding of rstd"):
            nc.vector.tensor_mul(out_ap, cur, cc[:])
        cur = dd[:]
    with nc.allow_low_precision(reason="fp32r rounding of rstd"):
        nc.vector.tensor_scalar(out=_r(pk[:, 1:2]), in0=stat[:, 0:1],
                                scalar1=pk[:, 0:1], scalar2=-1.0,
                                op0=ALU.mult, op1=ALU.mult)

    bc = p_sm.tile([128, 2], F32, tag="bc")
    if F_PBC:
        nc.gpsimd.partition_broadcast(bc[:], pk[:])
    else:
        bcp = psSm.tile([128, 2], F32, tag="sm")
        nc.tensor.matmul(bcp[:], _r(ones_row[:]), _r(pk[:]), start=True, stop=True)
        nc.vector.tensor_copy(bc[:], bcp[:])

    if F_ACTN:
        nc.scalar.activation(_r(h0t[:]), u0[:], AF.Identity,
                             bias=bc[:, 1:2], scale=bc[:, 0:1])
    else:
        nc.vector.tensor_scalar(out=_r(h0t[:]), in0=u0[:], scalar1=bc[:, 0:1],
                                scalar2=bc[:, 1:2], op0=ALU.mult, op1=ALU.add)
    nc.vector.tensor_scalar(out=_r(h1t[0:D1, :]), in0=u1[:], scalar1=bc[0:D1, 0:1],
                            scalar2=bc[0:D1, 1:2], op0=ALU.mult, op1=ALU.add)
